# revision 8
# baseline (speedup 1.0000x reference)
"""Chamfer distance on 8 TRN2 NeuronCores.

Problem: x [4, 3, 4096], y [4, 3, 4096] f32.
  dist[b, n, m] = sum_d (x[b,d,n] - y[b,d,m])^2
  out = mean_b( sum_n min_m dist + sum_m min_n dist )

Strategy (v2, rebalanced from the 70.8us baseline):
  - Shard: core c handles batch b = c//2, n-half h = c%2 (2048 rows x 4096 cols
    of the distance matrix per core).
  - dist = |x|^2 + |y|^2 - 2 x.y as a K=24 bf16 matmul per strip (Dekker
    triple-split on host, fp32 PSUM accumulate inside the PE array).
  - Evac: PSUM f32 strips [128, 2048] -> SBUF f16, mostly on ACT (1.89us/strip)
    with a tunable subset on DVE tensor_copy (2.26us/strip, 1x) to balance the
    two engines; extra DVE evacs placed in the ramp where DVE is otherwise
    idle.
  - Col chain split into TWO independent running-min accumulators (even tiles
    -> colaccA, odd -> colaccB; t0/t1 initialize them by evacuating straight
    into the accumulator). Host min-combines the four chain results per batch
    and folds partitions. Chain A ends at t14, so its cmin DMA overlaps t15.
  - Row mins: tiles 0-12 ship their full 4096-wide f16 strips to DRAM (DMA had
    ~25us slack in the baseline; this deletes the per-tile L1 tree from DVE).
    Tail tiles tree down (t13 -> 2048, t14 -> 1024, t15 -> 512) so the final
    DMAs are tiny. Host finishes all row mins + sums.
"""

import numpy as np
import ml_dtypes
from contextlib import ExitStack

import concourse.bass as bass
import concourse.mybir as mybir
import concourse.tile as tile
from concourse import bacc
from concourse.bass import ts, ds
from concourse.bass_utils import run_bass_kernel_spmd

B, D, N, M = 4, 3, 4096, 4096
NCORES = 8
HALF = N // 2            # rows of the distance matrix per core
NT = HALF // 128         # 16 row tiles per core
KROWS = 24               # contraction rows of the lifted matmul

# per-tile rmin width shipped to the host (tree depth = log2(4096/w))
STOPW = [4096] * 13 + [2048, 1024, 512]
# strips evacuated by DVE instead of ACT: set of (tile, strip_index)
DVE_EVAC = {(0, 2), (1, 1), (4, 1), (7, 1), (10, 1), (13, 1)}
T0_WIDTHS = [512, 1536, 2048]
T15_WIDTHS = [2048, 1024, 1024]
# first tile of each col chain (that tile evac-initializes the accumulator)
CHAIN_STARTS = [0, 8]
WARMUP_MM = 4

bf16 = ml_dtypes.bfloat16

# stash of the last BassKernelResults (test.py reads this)
last_results = None
_NC_CACHE = {}


def build_nc(reps: int = 1, cfg: dict | None = None) -> bass.Bass:
    cfg = cfg or {}
    stopw = cfg.get("STOPW", STOPW)
    dve_evac = cfg.get("DVE_EVAC", DVE_EVAC)
    t0_widths = cfg.get("T0_WIDTHS", T0_WIDTHS)
    t15_widths = cfg.get("T15_WIDTHS", T15_WIDTHS)
    chain_starts = cfg.get("CHAIN_STARTS", CHAIN_STARTS)
    warmup = cfg.get("WARMUP_MM", WARMUP_MM)
    nch = len(chain_starts)

    nc = bacc.Bacc()
    f32 = mybir.dt.float32
    f16 = mybir.dt.float16
    bft = mybir.dt.bfloat16
    mn = mybir.AluOpType.min

    def chain_of(t):
        ci = 0
        for i, s in enumerate(chain_starts):
            if t >= s:
                ci = i
        return ci

    chain_last = {}
    for t in range(NT):
        chain_last[chain_of(t)] = t

    # packed operand layout: ops = [lhsT_t0 (128) | rhs (M) | lhsT rest].
    OPS_W = HALF + M
    ops_d = nc.declare_dram_parameter("ops", [KROWS, OPS_W], bft, isOutput=False)
    rmin_d = nc.declare_dram_parameter("rmin", [128, NT, M], f16, isOutput=True)
    cmin_dn = [
        nc.declare_dram_parameter(f"cmin{i}", [128, M], f16, isOutput=True)
        for i in range(nch)
    ]

    with tile.TileContext(nc) as tc, ExitStack() as ctx:
        consts = ctx.enter_context(tc.tile_pool(name="consts", bufs=1))
        cp_pool = ctx.enter_context(tc.tile_pool(name="cp", bufs=4))
        ps_pool = ctx.enter_context(tc.tile_pool(name="ps", bufs=2, space="PSUM"))

        ops_sb = consts.tile([KROWS, OPS_W], bft)
        # lhsT column for tile t: t=0 lives at [0:128]; t>=1 at [128+M+128(t-1)]
        lhsT_col = lambda t: ops_sb[:, ts(0 if t == 0 else (M // 128) + t, 128)]
        rhs_sb = ops_sb[:, 128 : 128 + M]
        dummy = consts.tile([KROWS, 512], bft)  # uninitialized warmup operands

        # PE pstate warmup: garbage matmuls burn through the cold/mid clock
        # ramp while the operand DMA is still in flight
        if warmup:
            nc.gpsimd.memset(dummy[:, :], 0.0)
            wpd = ps_pool.tile([128, 2048], f32, tag="pd")
            for _ in range(warmup):
                nc.tensor.matmul(
                    wpd[:, 0:512], dummy[:, 0:128], dummy[:, 0:512],
                    start=True, stop=True,
                )

        nc.sync.dma_start(out=ops_sb[:, 0:640], in_=ops_d[:, 0:640])
        nc.sync.dma_start(out=ops_sb[:, 640:2176], in_=ops_d[:, 640:2176])
        nc.sync.dma_start(out=ops_sb[:, 2176:4224], in_=ops_d[:, 2176:4224])
        nc.sync.dma_start(out=ops_sb[:, 4224:OPS_W], in_=ops_d[:, 4224:OPS_W])

        coln = [
            consts.tile([128, M], f16, name=f"col{i}") for i in range(nch)
        ]

        for rep in range(reps):
            for t in range(NT):
                ci = chain_of(t)
                acc = coln[ci]
                is_init = t in chain_starts
                w = stopw[t]
                depth = {4096: 0, 2048: 1, 1024: 2, 512: 3}[w]
                cpg = acc if is_init else cp_pool.tile([128, M], f16, tag="cp")
                if t == 0:
                    widths = t0_widths
                elif t == NT - 1:
                    widths = t15_widths
                else:
                    widths = [2048, 2048]
                # matmuls + evacs first (a DVE evac must precede the tile's
                # col TTs in the DVE queue: it is ready earlier and frees its
                # PSUM strip for the PE)
                off = 0
                strip_rng = []
                for si, sw in enumerate(widths):
                    g0 = off
                    strip_rng.append((g0, sw))
                    pd = ps_pool.tile([128, 2048], f32, tag="pd")
                    for j in range(sw // 512):
                        nc.tensor.matmul(
                            pd[:, ts(j, 512)],
                            lhsT_col(t),
                            rhs_sb[:, ds(g0 + j * 512, 512)],
                            start=True,
                            stop=True,
                        )
                    off += sw
                    if (t, si) in dve_evac:
                        nc.vector.tensor_copy(cpg[:, ds(g0, sw)], pd[:, 0:sw])
                    else:
                        nc.scalar.copy(cpg[:, ds(g0, sw)], pd[:, 0:sw])
                    # ship row data early for tree-less tiles
                    if depth == 0:
                        nc.sync.dma_start(
                            out=rmin_d[:, t : t + 1, ds(g0, sw)],
                            in_=cpg[:, ds(g0, sw)],
                        )
                # col-chain updates (init tiles ARE the init via evac)
                for g0, sw in strip_rng:
                    if not is_init:
                        nc.vector.tensor_tensor(
                            out=acc[:, ds(g0, sw)],
                            in0=acc[:, ds(g0, sw)],
                            in1=cpg[:, ds(g0, sw)],
                            op=mn,
                        )
                    # chain-final: ship the accumulator per strip range
                    if t == chain_last[ci]:
                        nc.sync.dma_start(
                            out=cmin_dn[ci][:, ds(g0, sw)],
                            in_=acc[:, ds(g0, sw)],
                        )
                # row min tree down to stopw, then ship the prefix
                if depth > 0:
                    s = M // 2
                    for _ in range(depth):
                        nc.vector.tensor_tensor(
                            out=cpg[:, 0:s],
                            in0=cpg[:, 0:s],
                            in1=cpg[:, s : 2 * s],
                            op=mn,
                        )
                        s //= 2
                    nc.sync.dma_start(
                        out=rmin_d[:, t : t + 1, 0:w], in_=cpg[:, 0:w]
                    )

    nc.compile()
    return nc


def _get_nc(reps: int = 1) -> bass.Bass:
    if reps not in _NC_CACHE:
        _NC_CACHE[reps] = build_nc(reps)
    return _NC_CACHE[reps]


def _split3(v: np.ndarray):
    """Split float64 array into three bf16 terms summing to v (err ~2^-27|v|)."""
    a = v.astype(bf16)
    r = v - a.astype(np.float64)
    b = r.astype(bf16)
    r2 = r - b.astype(np.float64)
    c = r2.astype(bf16)
    return a, b, c


def build_operands(xs: np.ndarray, ys: np.ndarray):
    """Lift one core's shard into the K=24 bf16 matmul operands.

    xs: [3, HALF] f32 (x coords of this core's rows)
    ys: [3, M] f32 (full y for this batch)
    Returns lhsT [24, HALF] bf16, rhs [24, M] bf16 with
      (lhsT.T @ rhs)[n, m] ~= |x_n|^2 + |y_m|^2 - 2 x_n . y_m
    """
    xs64 = xs.astype(np.float64)
    ys64 = ys.astype(np.float64)
    u = -2.0 * xs64
    xsq = (xs64 * xs64).sum(axis=0)
    ysq = (ys64 * ys64).sum(axis=0)

    uh, um, ul = _split3(u)      # [3, HALF] each
    vh, vm, vl = _split3(ys64)   # [3, M] each
    xqh, xqm, xql = _split3(xsq)
    yqh, yqm, yql = _split3(ysq)
    ones_l = np.ones(HALF, dtype=bf16)
    ones_m = np.ones(M, dtype=bf16)

    lhs_rows, rhs_rows = [], []
    for d in range(D):
        for a, b_ in ((uh, vh), (uh, vm), (uh, vl), (um, vh), (um, vm), (ul, vh)):
            lhs_rows.append(a[d])
            rhs_rows.append(b_[d])
    for yq in (yqh, yqm, yql):
        lhs_rows.append(ones_l)
        rhs_rows.append(yq)
    for xq in (xqh, xqm, xql):
        lhs_rows.append(xq)
        rhs_rows.append(ones_m)

    lhsT = np.ascontiguousarray(np.stack(lhs_rows))
    rhs = np.ascontiguousarray(np.stack(rhs_rows))
    assert lhsT.shape == (KROWS, HALF) and rhs.shape == (KROWS, M)
    return lhsT, rhs


def make_in_maps(x: np.ndarray, y: np.ndarray):
    in_maps = []
    for c in range(NCORES):
        b, h = divmod(c, 2)
        lhsT, rhs = build_operands(x[b][:, h * HALF : (h + 1) * HALF], y[b])
        # packed layout: [lhsT tile0 | rhs | lhsT tiles 1..]
        ops = np.concatenate([lhsT[:, 0:128], rhs, lhsT[:, 128:]], axis=1)
        in_maps.append({"ops": np.ascontiguousarray(ops)})
    return in_maps


def combine_results(results):
    totals = []
    for b in range(B):
        r0 = results[2 * b]
        r1 = results[2 * b + 1]
        xsum = 0.0
        for r in (r0, r1):
            rm = np.asarray(r["rmin"], np.float64)  # [128, NT, M]
            for t in range(NT):
                xsum += rm[:, t, 0 : STOPW[t]].min(axis=1).sum()
        cm = np.minimum.reduce(
            [
                np.asarray(r[f"cmin{i}"], np.float64)
                for r in (r0, r1)
                for i in range(len(CHAIN_STARTS))
            ]
        )  # [128, M]
        totals.append(xsum + cm.min(axis=0).sum())
    return np.float32(np.mean(totals))


def kernel(x: np.ndarray, y: np.ndarray) -> np.ndarray:
    global last_results
    x = np.asarray(x, dtype=np.float32)
    y = np.asarray(y, dtype=np.float32)
    assert x.shape == (B, D, N) and y.shape == (B, D, M)
    in_maps = make_in_maps(x, y)
    res = run_bass_kernel_spmd(_get_nc(), in_maps, list(range(NCORES)))
    last_results = res
    return combine_results(res.results)
